# revision 21
# baseline (speedup 1.0000x reference)
"""Batch 3D-Gaussian rasterizer on 8 Trainium2 NeuronCores.

Strategy: host does the cheap per-gaussian preprocess (projection, conic,
SH color, depth sort) and tile culling; the device does the heavy
pixel x gaussian rasterization. The image is cut into 16x32-pixel tiles;
each (tile, 128-gaussian depth block) becomes one independent job that
computes per-pixel (color partial, invdepth partial, block transmittance).
Jobs are distributed round-robin over the 8 cores; the host then combines
a tile's blocks in depth order with C = C0 + T0*C1 + ..., which is exact.

Per-job device pipeline (gaussian axis on SBUF partitions, pixels on free):
  power  = coef^T @ F            (PE, K=6 quadratic-form features)
  alpha  = exp(power)            (ACT; ln(opacity) folded into coef const)
  alpha_c= min(alpha,.99)*(alpha>=1/255)   (DVE: min, is_ge, mult)
  l      = ln(1 - alpha_c)       (ACT, scale=-1 bias=1 fused)
  cum    = U_incl^T @ l          (PE, triangular-ones inclusive cumsum)
  E      = exp(cum)              (ACT)  # E[g] = T_after(g); T_before(g)=E[g-1]
  out4  += rgbd^T @ T_before - rgbd^T @ E   (PE, signed matmuls; w = Tb-Ta)
  out    = [out4 rows; E[127] = block transmittance row]
"""
import sys
import numpy as np

for _p in ('/opt/trn_rl_repo',):
    if _p not in sys.path:
        sys.path.append(_p)

B, P, H, W = 2, 1024, 128, 128
SCALE_MOD = 1.0
NEAR = 0.2
TS_Y, TS_X = 16, 16          # image tile = 16x16 = 256 px
NPIX = TS_Y * TS_X
GB = 128                     # SBUF partition dim (16 groups of GRAN rows)
GRAN = 8                     # gaussians per group (tile depth sub-block)
NGRP = GB // GRAN            # groups per job
MOUT = NGRP * 5              # output rows: 4 accums + 1 T row per group
NCORES = 8
NEG_BIG = -1.0e30

_C0 = 0.28209479177387814
_C1 = 0.4886025119029199
_C2 = (1.0925484305920792, -1.0925484305920792, 0.31539156525252005,
       -1.0925484305920792, 0.5462742152960396)
_C3 = (-0.5900435899266435, 2.890611442640554, -0.4570457994644658,
       0.3731763325901154, -0.4570457994644658, 1.445305721320277,
       -0.5900435899266435)


def _eval_sh(sh, dirs):
    x, y, z = dirs[:, 0:1], dirs[:, 1:2], dirs[:, 2:3]
    res = _C0 * sh[:, 0]
    res = res - _C1 * y * sh[:, 1] + _C1 * z * sh[:, 2] - _C1 * x * sh[:, 3]
    xx, yy, zz = x * x, y * y, z * z
    xy, yz, xz = x * y, y * z, x * z
    res = (res + _C2[0] * xy * sh[:, 4] + _C2[1] * yz * sh[:, 5]
           + _C2[2] * (2.0 * zz - xx - yy) * sh[:, 6]
           + _C2[3] * xz * sh[:, 7] + _C2[4] * (xx - yy) * sh[:, 8])
    res = (res + _C3[0] * y * (3.0 * xx - yy) * sh[:, 9]
           + _C3[1] * xy * z * sh[:, 10]
           + _C3[2] * y * (4.0 * zz - xx - yy) * sh[:, 11]
           + _C3[3] * z * (2.0 * zz - 3.0 * xx - 3.0 * yy) * sh[:, 12]
           + _C3[4] * x * (4.0 * zz - xx - yy) * sh[:, 13]
           + _C3[5] * z * (xx - yy) * sh[:, 14]
           + _C3[6] * x * (xx - 3.0 * yy) * sh[:, 15])
    return np.maximum(res + 0.5, 0.0).astype(np.float32)


def _quat_to_rot(q):
    q = q / np.linalg.norm(q, axis=-1, keepdims=True)
    r, x, y, z = q[:, 0], q[:, 1], q[:, 2], q[:, 3]
    return np.stack([
        1 - 2 * (y * y + z * z), 2 * (x * y - r * z), 2 * (x * z + r * y),
        2 * (x * y + r * z), 1 - 2 * (x * x + z * z), 2 * (y * z - r * x),
        2 * (x * z - r * y), 2 * (y * z + r * x), 1 - 2 * (x * x + y * y),
    ], axis=-1).reshape(-1, 3, 3).astype(np.float32)


def _preprocess_one(viewmat, projmat, campos, tanx, tany,
                    means3D, opacities, scales, rotations, sh):
    Pn = means3D.shape[0]
    ones = np.ones((Pn, 1), means3D.dtype)
    p_hom = np.concatenate([means3D, ones], axis=1)
    t = (p_hom @ viewmat)[:, :3]
    p_proj = p_hom @ projmat
    ndc = p_proj[:, :3] / (p_proj[:, 3:4] + 1e-7)
    depth = t[:, 2]

    focal_x = W / (2.0 * tanx)
    focal_y = H / (2.0 * tany)
    tz = np.maximum(depth, 1e-6)
    txc = np.clip(t[:, 0] / tz, -1.3 * tanx, 1.3 * tanx) * tz
    tyc = np.clip(t[:, 1] / tz, -1.3 * tany, 1.3 * tany) * tz
    zero = np.zeros_like(tz)
    J = np.stack([
        np.stack([focal_x / tz, zero, -focal_x * txc / (tz * tz)], -1),
        np.stack([zero, focal_y / tz, -focal_y * tyc / (tz * tz)], -1),
    ], axis=1).astype(np.float32)
    Wr = viewmat[:3, :3].T
    Rq = _quat_to_rot(rotations)
    s2 = ((scales * SCALE_MOD) ** 2).astype(np.float32)
    Sigma = np.einsum('pij,pj,pkj->pik', Rq, s2, Rq).astype(np.float32)
    T = np.einsum('pij,jk->pik', J, Wr).astype(np.float32)
    cov2d = np.einsum('pij,pjk,plk->pil', T, Sigma, T).astype(np.float32)
    a = cov2d[:, 0, 0] + 0.3
    c = cov2d[:, 1, 1] + 0.3
    b = cov2d[:, 0, 1]
    det = a * c - b * b
    valid = (det > 0.0) & (depth > NEAR)
    det_s = np.where(valid, det, 1.0)
    conA, conB, conC = c / det_s, -b / det_s, a / det_s

    mid = 0.5 * (a + c)
    lam = mid + np.sqrt(np.maximum(0.1, mid * mid - det))
    radii = np.where(valid, np.ceil(3.0 * np.sqrt(lam)), 0.0).astype(np.int32)

    px = ((ndc[:, 0] + 1.0) * W - 1.0) * 0.5
    py = ((ndc[:, 1] + 1.0) * H - 1.0) * 0.5

    dirs = means3D - campos[None, :]
    dirs = dirs / np.linalg.norm(dirs, axis=-1, keepdims=True)
    rgb = _eval_sh(sh, dirs)

    order = np.argsort(depth, kind='stable')
    return dict(
        px=px[order], py=py[order],
        cA=conA[order], cB=conB[order], cC=conC[order],
        op=opacities[order, 0], rgb=rgb[order],
        dep=np.maximum(depth[order], 1e-6), valid=valid[order],
        lam=lam[order], radii_unsorted=radii,
    )


def _build_groups(pps):
    """Cut every (cam, tile) gaussian list into depth-ordered chunks of
    GRAN. Returns a list of group dicts: coef [6,GRAN], wd [GRAN,5]
    (diff-rgbd cols 0:4, T-indicator col 4), rfirst [4] (host-added
    constant term), and (b, ty, tx, chunk)."""
    nty, ntx = H // TS_Y, W // TS_X
    groups = []
    for b, pp in enumerate(pps):
        op = pp['op'].astype(np.float64)
        ln_t = np.log(np.maximum(op * 255.0, 1e-300))
        r = np.sqrt(2.0 * np.maximum(ln_t, 0.0) * pp['lam'])
        r = np.where((ln_t > 0) & pp['valid'], r, 0.0) + 0.5
        live = pp['valid'] & (ln_t > 0)
        px, py = pp['px'].astype(np.float64), pp['py'].astype(np.float64)
        A = pp['cA'].astype(np.float64)
        Bc = pp['cB'].astype(np.float64)
        C = pp['cC'].astype(np.float64)
        lnop = np.log(np.maximum(op, 1e-300))
        rgbd4 = np.zeros((P, 4), np.float32)
        rgbd4[:, 0:3] = pp['rgb']
        rgbd4[:, 3] = np.float32(1.0) / pp['dep']
        for ty in range(nty):
            y0, y1 = ty * TS_Y, (ty + 1) * TS_Y - 1
            for tx in range(ntx):
                x0, x1 = tx * TS_X, (tx + 1) * TS_X - 1
                bbox = (live & (px + r >= x0) & (px - r <= x1)
                        & (py + r >= y0) & (py - r <= y1))
                # exact max of the (concave) power quadratic over the
                # tile rect; conservative vs the integer pixel grid
                dxl, dxh = x0 - px, x1 - px
                dyl, dyh = y0 - py, y1 - py
                inside = (dxl <= 0) & (dxh >= 0) & (dyl <= 0) & (dyh >= 0)
                best = np.where(inside, 0.0, -np.inf)
                for dx in (dxl, dxh):
                    ys = np.clip(-Bc * dx / C, dyl, dyh)
                    best = np.maximum(best, -0.5 * (A * dx * dx + C * ys * ys)
                                      - Bc * dx * ys)
                for dy in (dyl, dyh):
                    xs = np.clip(-Bc * dy / A, dxl, dxh)
                    best = np.maximum(best, -0.5 * (A * xs * xs + C * dy * dy)
                                      - Bc * xs * dy)
                lnth = np.log(1.0 / 255.0) - lnop
                sel = np.nonzero(bbox & (best >= lnth - 1e-3))[0]
                if sel.size == 0:
                    continue
                cx = x0 + (TS_X - 1) / 2.0
                cy = y0 + (TS_Y - 1) / 2.0
                pxl, pyl = px[sel] - cx, py[sel] - cy
                As, Bs, Cs = A[sel], Bc[sel], C[sel]
                c6 = np.stack([
                    -0.5 * As,
                    -0.5 * Cs,
                    -Bs,
                    As * pxl + Bs * pyl,
                    Cs * pyl + Bs * pxl,
                    -(0.5 * As * pxl * pxl + 0.5 * Cs * pyl * pyl
                      + Bs * pxl * pyl) + lnop[sel],
                ], axis=0)  # [6, n]
                n = sel.size
                for k in range((n + GRAN - 1) // GRAN):
                    s, e = k * GRAN, min((k + 1) * GRAN, n)
                    m = e - s
                    ct = np.zeros((6, GRAN), np.float32)
                    ct[5, :] = NEG_BIG
                    ct[:, :m] = c6[:, s:e].astype(np.float32)
                    r4 = np.zeros((GRAN, 4), np.float32)
                    r4[:m] = rgbd4[sel[s:e]]
                    wd = np.zeros((GRAN, 5), np.float32)
                    wd[0:GRAN - 1, 0:4] = r4[1:GRAN] - r4[0:GRAN - 1]
                    wd[GRAN - 1, 0:4] = -r4[GRAN - 1]
                    wd[GRAN - 1, 4] = 1.0   # T row: picks E[last]
                    groups.append(dict(key=(b, ty, tx, k), coef=ct, wd=wd,
                                       rfirst=r4[0].copy()))
    return groups


def _features():
    yy, xx = np.meshgrid(np.arange(TS_Y), np.arange(TS_X), indexing='ij')
    xl = (xx - (TS_X - 1) / 2.0).ravel()
    yl = (yy - (TS_Y - 1) / 2.0).ravel()
    F = np.stack([xl * xl, yl * yl, xl * yl, xl, yl,
                  np.ones(NPIX)], axis=0).astype(np.float32)  # [6, NPIX]
    # block-diag [12, 2*NPIX] so one matmul computes a pair of jobs
    F2 = np.zeros((12, 2 * NPIX), np.float32)
    F2[0:6, 0:NPIX] = F
    F2[6:12, NPIX:2 * NPIX] = F
    return F2


_NC_CACHE = {}


def _build_nc(J):
    if J in _NC_CACHE:
        return _NC_CACHE[J]
    import concourse.bacc as bacc
    import concourse.mybir as mybir
    import concourse.tile as tile
    from concourse.tile import add_dep_helper
    from contextlib import ExitStack

    f32 = mybir.dt.float32
    f32r = mybir.dt.float32r
    Alu = mybir.AluOpType
    Act = mybir.ActivationFunctionType

    nc = bacc.Bacc("TRN2", target_bir_lowering=False, debug=False,
                   num_devices=NCORES)
    # batched layouts: one DMA each for coef / wout / out
    coef_d = nc.dram_tensor("coef", [(J + 1) // 2, 12, GB], f32, kind="ExternalInput")
    wout_d = nc.dram_tensor("wout", [J, GB, MOUT], f32, kind="ExternalInput")
    F_d = nc.dram_tensor("feat", [12, 2 * NPIX], f32, kind="ExternalInput")
    U_d = nc.dram_tensor("triu", [GB, GB], f32, kind="ExternalInput")
    out_d = nc.dram_tensor("out", [MOUT, J, NPIX], f32, kind="ExternalOutput")

    with tile.TileContext(nc) as tc, ExitStack() as ctx:
        pairs = [list(range(p, min(p + 2, J))) for p in range(0, J, 2)]
        NP = len(pairs)
        const = ctx.enter_context(tc.tile_pool(name="const", bufs=1))
        pool = ctx.enter_context(tc.tile_pool(name="work", bufs=3))
        lpool = ctx.enter_context(tc.tile_pool(name="lpool", bufs=NP))
        epool = ctx.enter_context(tc.tile_pool(name="epool", bufs=NP))
        acpool = ctx.enter_context(tc.tile_pool(name="acpool", bufs=NP))
        psum = ctx.enter_context(
            tc.tile_pool(name="psum", bufs=2, space="PSUM"))

        F_t = const.tile([12, 2 * NPIX], f32)
        nc.sync.dma_start(F_t[:], F_d[:])
        coef_t = const.tile([12, ((J + 1) // 2) * GB], f32)
        nc.sync.dma_start(
            coef_t[:].rearrange("p (j g) -> p j g", j=(J + 1) // 2),
            coef_d[:].rearrange("j p g -> p j g"))
        U_t = const.tile([GB, GB], f32)
        nc.gpsimd.dma_start(U_t[:], U_d[:])
        wout_t = const.tile([GB, J * MOUT], f32)
        nc.gpsimd.dma_start(
            wout_t[:].rearrange("p (j m) -> p j m", j=J),
            wout_d[:].rearrange("j p m -> p j m"))
        oall_t = const.tile([MOUT, J * NPIX], f32)

        alphas, acs, ls, es = ({} for _ in range(4))
        expA, lns, expC = [], [], []
        # phase A: feature matmuls (per job) + one exp per pair
        for p, pr in enumerate(pairs):
            wN = len(pr) * NPIX
            power_p = psum.tile([GB, 512], f32, tag="power",
                                name=f"power{p}")
            nc.tensor.matmul(power_p[:, 0:wN],
                             coef_t[:, p * GB:(p + 1) * GB],
                             F_t[:, 0:wN], start=True, stop=True)
            alphas[p] = pool.tile([GB, wN], f32, tag="alpha",
                                  name=f"alpha{p}")
            expA.append(nc.scalar.activation(alphas[p][:],
                                             power_p[:, 0:wN], Act.Exp))
        # phase B: DVE clamp+mask per pair, then Ln per pair
        prev_dve = None
        for p, pr in enumerate(pairs):
            wN = len(pr) * NPIX
            amin_t = pool.tile([GB, wN], f32, tag="amin")
            i0 = nc.vector.tensor_scalar(amin_t[:], alphas[p][:], 0.99, None,
                                         Alu.min)
            if prev_dve is not None:
                add_dep_helper(i0.ins, prev_dve.ins, sync=False,
                               reason="keep per-pair DVE chain contiguous")
            mask_t = pool.tile([GB, wN], f32, tag="mask")
            nc.vector.tensor_scalar(mask_t[:], alphas[p][:], 1.0 / 255.0,
                                    None, Alu.is_ge)
            acs[p] = acpool.tile([GB, wN], f32, tag="ac", name=f"ac{p}")
            prev_dve = nc.vector.tensor_tensor(acs[p][:], amin_t[:],
                                               mask_t[:], Alu.mult)
        for p, pr in enumerate(pairs):
            wN = len(pr) * NPIX
            ls[p] = lpool.tile([GB, wN], f32, tag="l", name=f"l{p}")
            i = nc.scalar.activation(ls[p][:], acs[p][:], Act.Ln, bias=1.0,
                                     scale=-1.0)
            lns.append(i)
            add_dep_helper(i.ins, expA[-1].ins, sync=False,
                           reason="group ACT tables: Ln after phase-A Exps")
        # phase C: cumsum matmul + exp per pair
        for p, pr in enumerate(pairs):
            wN = len(pr) * NPIX
            cum_p = psum.tile([GB, 512], f32, tag="cum", name=f"cum{p}")
            nc.tensor.matmul(cum_p[:, 0:wN], U_t[:], ls[p][:],
                             start=True, stop=True)
            es[p] = epool.tile([GB, wN], f32, tag="e", name=f"e{p}")
            i = nc.scalar.activation(es[p][:], cum_p[:, 0:wN], Act.Exp)
            expC.append(i)
            add_dep_helper(i.ins, lns[-1].ins, sync=False,
                           reason="group ACT tables: Exp after all Ln")
        # phase D: per-job output matmul, copy into staging, DMA per pair
        od = out_d[:].rearrange("p j n -> p (j n)")
        for p, pr in enumerate(pairs):
            for o, j in enumerate(pr):
                out_p = psum.tile([MOUT, NPIX], f32, tag="out")
                nc.tensor.matmul(out_p[:],
                                 wout_t[:, j * MOUT:(j + 1) * MOUT],
                                 es[p][:, o * NPIX:(o + 1) * NPIX],
                                 start=True, stop=True)
                nc.vector.tensor_copy(oall_t[:, j * NPIX:(j + 1) * NPIX],
                                      out_p[:])
            j0, j1 = pr[0], pr[-1] + 1
            nc.sync.dma_start(od[:, j0 * NPIX:j1 * NPIX],
                              oall_t[:, j0 * NPIX:j1 * NPIX])

    nc.compile()
    _NC_CACHE[J] = nc
    return nc


def kernel(means3D, opacities, scales, rotations, sh, bg,
           viewmatrices, projmatrices, camposes, tanfovxs, tanfovys,
           _run_opts=None):
    from concourse.bass_utils import run_bass_kernel_spmd

    means3D = np.asarray(means3D, np.float32)
    opacities = np.asarray(opacities, np.float32)
    scales = np.asarray(scales, np.float32)
    rotations = np.asarray(rotations, np.float32)
    sh = np.asarray(sh, np.float32)
    bg = np.asarray(bg, np.float32)
    viewmatrices = np.asarray(viewmatrices, np.float32)
    projmatrices = np.asarray(projmatrices, np.float32)
    camposes = np.asarray(camposes, np.float32)
    tanfovxs = np.asarray(tanfovxs, np.float32)
    tanfovys = np.asarray(tanfovys, np.float32)

    pps = [_preprocess_one(viewmatrices[b], projmatrices[b], camposes[b],
                           float(tanfovxs[b]), float(tanfovys[b]),
                           means3D, opacities, scales, rotations, sh)
           for b in range(B)]
    groups = _build_groups(pps)

    njob = (len(groups) + NGRP - 1) // NGRP
    J = (njob + NCORES - 1) // NCORES
    # pad with dummy groups (alpha=0 everywhere, no output consumer)
    dummy = dict(key=None, coef=None, wd=None, rfirst=None)
    while len(groups) < J * NCORES * NGRP:
        groups.append(dummy)

    F = _features()
    U8 = np.triu(np.ones((GRAN, GRAN), np.float32))
    U = np.zeros((GB, GB), np.float32)
    for g in range(NGRP):
        U[g * GRAN:(g + 1) * GRAN, g * GRAN:(g + 1) * GRAN] = U8

    # pack per (core, pair): coef [12,GB] (rows 0:6 job a, 6:12 job b)
    NPAIR = (J + 1) // 2
    coef_all = np.zeros((NCORES, NPAIR, 12, GB), np.float32)
    coef_all[:, :, 5, :] = NEG_BIG
    coef_all[:, :, 11, :] = NEG_BIG
    wout_all = np.zeros((NCORES, J, GB, MOUT), np.float32)
    gmeta = {}  # (core, j, g) -> (key, rfirst)
    for i, gr in enumerate(groups):
        cj, g = divmod(i, NGRP)
        c, j = divmod(cj, J)
        if gr['key'] is None:
            continue
        p, half = divmod(j, 2)
        coef_all[c, p, half * 6:(half + 1) * 6,
                 g * GRAN:(g + 1) * GRAN] = gr['coef']
        wout_all[c, j, g * GRAN:(g + 1) * GRAN, g * 5:(g + 1) * 5] = gr['wd']
        gmeta[(c, j, g)] = (gr['key'], gr['rfirst'])

    nc = _build_nc(J)
    in_maps = [{'coef': coef_all[c], 'wout': wout_all[c],
                'feat': F, 'triu': U} for c in range(NCORES)]
    run_opts = dict(_run_opts or {})
    result_sink = run_opts.pop('result_sink', None)
    res = run_bass_kernel_spmd(nc, in_maps, list(range(NCORES)), **run_opts)
    if result_sink is not None:
        result_sink['res'] = res
    outs = [res.results[c]['out'] for c in range(NCORES)]  # [MOUT,J,NPIX]

    # host combine: per (cam, tile) chain chunks in depth order
    nty, ntx = H // TS_Y, W // TS_X
    acc = {}
    for (c, j, g), (key, rfirst) in gmeta.items():
        b, ty, tx, chunk = key
        o = outs[c][g * 5:(g + 1) * 5, j]          # [5, NPIX]
        acc.setdefault((b, ty, tx), []).append((chunk, o, rfirst))
    colors = np.zeros((B, 3, H, W), np.float32)
    invd = np.zeros((B, 1, H, W), np.float32)
    for b in range(B):
        colors[b] = bg[:, None, None]
    for (b, ty, tx), chunks in acc.items():
        chunks.sort(key=lambda t: t[0])
        Csum = np.zeros((4, NPIX), np.float32)
        Trun = np.ones((NPIX,), np.float32)
        for _, o, rfirst in chunks:
            Csum = Csum + Trun[None, :] * (o[0:4] + rfirst[:, None])
            Trun = Trun * o[4]
        ybase, xbase = ty * TS_Y, tx * TS_X
        tilec = (Csum[0:3] + Trun[None, :] * bg[:, None]).reshape(
            3, TS_Y, TS_X)
        colors[b, :, ybase:ybase + TS_Y, xbase:xbase + TS_X] = tilec
        invd[b, 0, ybase:ybase + TS_Y, xbase:xbase + TS_X] = \
            Csum[3].reshape(TS_Y, TS_X)

    radii = np.stack([pp['radii_unsorted'] for pp in pps])
    return colors, invd, radii


# revision 22
# speedup vs baseline: 1.0030x; 1.0030x over previous
"""Batch 3D-Gaussian rasterizer on 8 Trainium2 NeuronCores.

Host does the cheap per-gaussian preprocess (projection, EWA conic, SH
color, depth sort) and exact per-tile culling; the device does the heavy
pixel x gaussian rasterization. The image is cut into 16x16-pixel tiles;
each (tile, 8-gaussian depth chunk) is one "group". 16 groups pack the
128 SBUF partitions of one job (tile identity lives in the host-packed
quadratic coefficients; the pixel-feature matrix F uses tile-local
coordinates so it is shared). Jobs spread round-robin over 8 cores; the
host chains each tile's chunks in depth order with the associative
combine C = C0 + T0*C1 + ..., which is exact.

Per-job device pipeline (gaussians on partitions, pixels on free axis):
  power = coef^T @ F2     (PE, K=12 block-diag: one matmul per job pair)
  alpha = exp(power)      (ACT; ln(opacity) folded into coef constant)
  ac    = min(alpha,.99) * (alpha >= 1/255)      (DVE min/is_ge/mult)
  l     = ln(1 - ac)      (ACT, scale=-1 bias=1 fused)
  cum   = Ublk^T @ l      (PE, block-diag triangular-ones cumsum)
  E     = exp(cum)        (ACT)  # E[g] = T_after(g) within the chunk
  out   = wd^T @ E        (PE; wd = row-diff of rgb/invdepth so that
                           w = T_before - T_after needs no extra ops;
                           an indicator column emits the chunk T row)
Activations are phase-grouped (add_dep) so ACT loads each table once;
all job inputs/outputs move in one batched DMA per tensor per queue.
"""
import sys
import numpy as np

for _p in ('/opt/trn_rl_repo',):
    if _p not in sys.path:
        sys.path.append(_p)

B, P, H, W = 2, 1024, 128, 128
SCALE_MOD = 1.0
NEAR = 0.2
TS_Y, TS_X = 16, 16          # image tile = 16x16 = 256 px
NPIX = TS_Y * TS_X
GB = 128                     # SBUF partition dim (16 groups of GRAN rows)
GRAN = 8                     # gaussians per group (tile depth sub-block)
NGRP = GB // GRAN            # groups per job
MOUT = NGRP * 5              # output rows: 4 accums + 1 T row per group
NCORES = 8
NEG_BIG = -1.0e30

_C0 = 0.28209479177387814
_C1 = 0.4886025119029199
_C2 = (1.0925484305920792, -1.0925484305920792, 0.31539156525252005,
       -1.0925484305920792, 0.5462742152960396)
_C3 = (-0.5900435899266435, 2.890611442640554, -0.4570457994644658,
       0.3731763325901154, -0.4570457994644658, 1.445305721320277,
       -0.5900435899266435)


def _eval_sh(sh, dirs):
    x, y, z = dirs[:, 0:1], dirs[:, 1:2], dirs[:, 2:3]
    res = _C0 * sh[:, 0]
    res = res - _C1 * y * sh[:, 1] + _C1 * z * sh[:, 2] - _C1 * x * sh[:, 3]
    xx, yy, zz = x * x, y * y, z * z
    xy, yz, xz = x * y, y * z, x * z
    res = (res + _C2[0] * xy * sh[:, 4] + _C2[1] * yz * sh[:, 5]
           + _C2[2] * (2.0 * zz - xx - yy) * sh[:, 6]
           + _C2[3] * xz * sh[:, 7] + _C2[4] * (xx - yy) * sh[:, 8])
    res = (res + _C3[0] * y * (3.0 * xx - yy) * sh[:, 9]
           + _C3[1] * xy * z * sh[:, 10]
           + _C3[2] * y * (4.0 * zz - xx - yy) * sh[:, 11]
           + _C3[3] * z * (2.0 * zz - 3.0 * xx - 3.0 * yy) * sh[:, 12]
           + _C3[4] * x * (4.0 * zz - xx - yy) * sh[:, 13]
           + _C3[5] * z * (xx - yy) * sh[:, 14]
           + _C3[6] * x * (xx - 3.0 * yy) * sh[:, 15])
    return np.maximum(res + 0.5, 0.0).astype(np.float32)


def _quat_to_rot(q):
    q = q / np.linalg.norm(q, axis=-1, keepdims=True)
    r, x, y, z = q[:, 0], q[:, 1], q[:, 2], q[:, 3]
    return np.stack([
        1 - 2 * (y * y + z * z), 2 * (x * y - r * z), 2 * (x * z + r * y),
        2 * (x * y + r * z), 1 - 2 * (x * x + z * z), 2 * (y * z - r * x),
        2 * (x * z - r * y), 2 * (y * z + r * x), 1 - 2 * (x * x + y * y),
    ], axis=-1).reshape(-1, 3, 3).astype(np.float32)


def _preprocess_one(viewmat, projmat, campos, tanx, tany,
                    means3D, opacities, scales, rotations, sh):
    Pn = means3D.shape[0]
    ones = np.ones((Pn, 1), means3D.dtype)
    p_hom = np.concatenate([means3D, ones], axis=1)
    t = (p_hom @ viewmat)[:, :3]
    p_proj = p_hom @ projmat
    ndc = p_proj[:, :3] / (p_proj[:, 3:4] + 1e-7)
    depth = t[:, 2]

    focal_x = W / (2.0 * tanx)
    focal_y = H / (2.0 * tany)
    tz = np.maximum(depth, 1e-6)
    txc = np.clip(t[:, 0] / tz, -1.3 * tanx, 1.3 * tanx) * tz
    tyc = np.clip(t[:, 1] / tz, -1.3 * tany, 1.3 * tany) * tz
    zero = np.zeros_like(tz)
    J = np.stack([
        np.stack([focal_x / tz, zero, -focal_x * txc / (tz * tz)], -1),
        np.stack([zero, focal_y / tz, -focal_y * tyc / (tz * tz)], -1),
    ], axis=1).astype(np.float32)
    Wr = viewmat[:3, :3].T
    Rq = _quat_to_rot(rotations)
    s2 = ((scales * SCALE_MOD) ** 2).astype(np.float32)
    Sigma = np.einsum('pij,pj,pkj->pik', Rq, s2, Rq).astype(np.float32)
    T = np.einsum('pij,jk->pik', J, Wr).astype(np.float32)
    cov2d = np.einsum('pij,pjk,plk->pil', T, Sigma, T).astype(np.float32)
    a = cov2d[:, 0, 0] + 0.3
    c = cov2d[:, 1, 1] + 0.3
    b = cov2d[:, 0, 1]
    det = a * c - b * b
    valid = (det > 0.0) & (depth > NEAR)
    det_s = np.where(valid, det, 1.0)
    conA, conB, conC = c / det_s, -b / det_s, a / det_s

    mid = 0.5 * (a + c)
    lam = mid + np.sqrt(np.maximum(0.1, mid * mid - det))
    radii = np.where(valid, np.ceil(3.0 * np.sqrt(lam)), 0.0).astype(np.int32)

    px = ((ndc[:, 0] + 1.0) * W - 1.0) * 0.5
    py = ((ndc[:, 1] + 1.0) * H - 1.0) * 0.5

    dirs = means3D - campos[None, :]
    dirs = dirs / np.linalg.norm(dirs, axis=-1, keepdims=True)
    rgb = _eval_sh(sh, dirs)

    order = np.argsort(depth, kind='stable')
    return dict(
        px=px[order], py=py[order],
        cA=conA[order], cB=conB[order], cC=conC[order],
        op=opacities[order, 0], rgb=rgb[order],
        dep=np.maximum(depth[order], 1e-6), valid=valid[order],
        lam=lam[order], radii_unsorted=radii,
    )


def _build_groups(pps):
    """Cut every (cam, tile) gaussian list into depth-ordered chunks of
    GRAN. Returns a list of group dicts: coef [6,GRAN], wd [GRAN,5]
    (diff-rgbd cols 0:4, T-indicator col 4), rfirst [4] (host-added
    constant term), and (b, ty, tx, chunk)."""
    nty, ntx = H // TS_Y, W // TS_X
    groups = []
    for b, pp in enumerate(pps):
        op = pp['op'].astype(np.float64)
        ln_t = np.log(np.maximum(op * 255.0, 1e-300))
        r = np.sqrt(2.0 * np.maximum(ln_t, 0.0) * pp['lam'])
        r = np.where((ln_t > 0) & pp['valid'], r, 0.0) + 0.5
        live = pp['valid'] & (ln_t > 0)
        px, py = pp['px'].astype(np.float64), pp['py'].astype(np.float64)
        A = pp['cA'].astype(np.float64)
        Bc = pp['cB'].astype(np.float64)
        C = pp['cC'].astype(np.float64)
        lnop = np.log(np.maximum(op, 1e-300))
        rgbd4 = np.zeros((P, 4), np.float32)
        rgbd4[:, 0:3] = pp['rgb']
        rgbd4[:, 3] = np.float32(1.0) / pp['dep']
        for ty in range(nty):
            y0, y1 = ty * TS_Y, (ty + 1) * TS_Y - 1
            for tx in range(ntx):
                x0, x1 = tx * TS_X, (tx + 1) * TS_X - 1
                bbox = (live & (px + r >= x0) & (px - r <= x1)
                        & (py + r >= y0) & (py - r <= y1))
                # exact max of the (concave) power quadratic over the
                # tile rect; conservative vs the integer pixel grid
                dxl, dxh = x0 - px, x1 - px
                dyl, dyh = y0 - py, y1 - py
                inside = (dxl <= 0) & (dxh >= 0) & (dyl <= 0) & (dyh >= 0)
                best = np.where(inside, 0.0, -np.inf)
                for dx in (dxl, dxh):
                    ys = np.clip(-Bc * dx / C, dyl, dyh)
                    best = np.maximum(best, -0.5 * (A * dx * dx + C * ys * ys)
                                      - Bc * dx * ys)
                for dy in (dyl, dyh):
                    xs = np.clip(-Bc * dy / A, dxl, dxh)
                    best = np.maximum(best, -0.5 * (A * xs * xs + C * dy * dy)
                                      - Bc * xs * dy)
                lnth = np.log(1.0 / 255.0) - lnop
                sel = np.nonzero(bbox & (best >= lnth - 1e-3))[0]
                if sel.size == 0:
                    continue
                cx = x0 + (TS_X - 1) / 2.0
                cy = y0 + (TS_Y - 1) / 2.0
                pxl, pyl = px[sel] - cx, py[sel] - cy
                As, Bs, Cs = A[sel], Bc[sel], C[sel]
                c6 = np.stack([
                    -0.5 * As,
                    -0.5 * Cs,
                    -Bs,
                    As * pxl + Bs * pyl,
                    Cs * pyl + Bs * pxl,
                    -(0.5 * As * pxl * pxl + 0.5 * Cs * pyl * pyl
                      + Bs * pxl * pyl) + lnop[sel],
                ], axis=0)  # [6, n]
                n = sel.size
                for k in range((n + GRAN - 1) // GRAN):
                    s, e = k * GRAN, min((k + 1) * GRAN, n)
                    m = e - s
                    ct = np.zeros((6, GRAN), np.float32)
                    ct[5, :] = NEG_BIG
                    ct[:, :m] = c6[:, s:e].astype(np.float32)
                    r4 = np.zeros((GRAN, 4), np.float32)
                    r4[:m] = rgbd4[sel[s:e]]
                    wd = np.zeros((GRAN, 5), np.float32)
                    wd[0:GRAN - 1, 0:4] = r4[1:GRAN] - r4[0:GRAN - 1]
                    wd[GRAN - 1, 0:4] = -r4[GRAN - 1]
                    wd[GRAN - 1, 4] = 1.0   # T row: picks E[last]
                    groups.append(dict(key=(b, ty, tx, k), coef=ct, wd=wd,
                                       rfirst=r4[0].copy()))
    return groups


def _features():
    yy, xx = np.meshgrid(np.arange(TS_Y), np.arange(TS_X), indexing='ij')
    xl = (xx - (TS_X - 1) / 2.0).ravel()
    yl = (yy - (TS_Y - 1) / 2.0).ravel()
    F = np.stack([xl * xl, yl * yl, xl * yl, xl, yl,
                  np.ones(NPIX)], axis=0).astype(np.float32)  # [6, NPIX]
    # block-diag [12, 2*NPIX] so one matmul computes a pair of jobs
    F2 = np.zeros((12, 2 * NPIX), np.float32)
    F2[0:6, 0:NPIX] = F
    F2[6:12, NPIX:2 * NPIX] = F
    return F2


_NC_CACHE = {}


def _build_nc(J):
    if J in _NC_CACHE:
        return _NC_CACHE[J]
    import concourse.bacc as bacc
    import concourse.mybir as mybir
    import concourse.tile as tile
    from concourse.tile import add_dep_helper
    from contextlib import ExitStack

    f32 = mybir.dt.float32
    f32r = mybir.dt.float32r
    Alu = mybir.AluOpType
    Act = mybir.ActivationFunctionType

    nc = bacc.Bacc("TRN2", target_bir_lowering=False, debug=False,
                   num_devices=NCORES)
    # batched layouts: one DMA each for coef / wout / out
    coef_d = nc.dram_tensor("coef", [(J + 1) // 2, 12, GB], f32, kind="ExternalInput")
    wout_d = nc.dram_tensor("wout", [J, GB, MOUT], f32, kind="ExternalInput")
    F_d = nc.dram_tensor("feat", [12, 2 * NPIX], f32, kind="ExternalInput")
    U_d = nc.dram_tensor("triu", [GB, GB], f32, kind="ExternalInput")
    out_d = nc.dram_tensor("out", [MOUT, J, NPIX], f32, kind="ExternalOutput")

    with tile.TileContext(nc) as tc, ExitStack() as ctx:
        pairs = [list(range(p, min(p + 2, J))) for p in range(0, J, 2)]
        NP = len(pairs)
        const = ctx.enter_context(tc.tile_pool(name="const", bufs=1))
        pool = ctx.enter_context(tc.tile_pool(name="work", bufs=3))
        lpool = ctx.enter_context(tc.tile_pool(name="lpool", bufs=NP))
        epool = ctx.enter_context(tc.tile_pool(name="epool", bufs=NP))
        acpool = ctx.enter_context(tc.tile_pool(name="acpool", bufs=NP))
        psum = ctx.enter_context(
            tc.tile_pool(name="psum", bufs=2, space="PSUM"))

        F_t = const.tile([12, 2 * NPIX], f32)
        nc.sync.dma_start(F_t[:], F_d[:])
        coef_t = const.tile([12, ((J + 1) // 2) * GB], f32)
        nc.sync.dma_start(
            coef_t[:].rearrange("p (j g) -> p j g", j=(J + 1) // 2),
            coef_d[:].rearrange("j p g -> p j g"))
        U_t = const.tile([GB, GB], f32)
        nc.gpsimd.dma_start(U_t[:], U_d[:])
        wout_t = const.tile([GB, J * MOUT], f32)
        nc.gpsimd.dma_start(
            wout_t[:].rearrange("p (j m) -> p j m", j=J),
            wout_d[:].rearrange("j p m -> p j m"))
        oall_t = const.tile([MOUT, J * NPIX], f32)

        alphas, acs, ls, es = ({} for _ in range(4))
        expA, lns, expC = [], [], []
        # phase A: feature matmuls (per job) + one exp per pair
        for p, pr in enumerate(pairs):
            wN = len(pr) * NPIX
            power_p = psum.tile([GB, 512], f32, tag="power",
                                name=f"power{p}")
            nc.tensor.matmul(power_p[:, 0:wN],
                             coef_t[:, p * GB:(p + 1) * GB],
                             F_t[:, 0:wN], start=True, stop=True)
            alphas[p] = pool.tile([GB, wN], f32, tag="alpha",
                                  name=f"alpha{p}")
            expA.append(nc.scalar.activation(alphas[p][:],
                                             power_p[:, 0:wN], Act.Exp))
        # phase B: DVE clamp+mask per pair, then Ln per pair
        prev_dve = None
        for p, pr in enumerate(pairs):
            wN = len(pr) * NPIX
            amin_t = pool.tile([GB, wN], f32, tag="amin")
            i0 = nc.vector.tensor_scalar(amin_t[:], alphas[p][:], 0.99, None,
                                         Alu.min)
            if prev_dve is not None:
                add_dep_helper(i0.ins, prev_dve.ins, sync=False,
                               reason="keep per-pair DVE chain contiguous")
            mask_t = pool.tile([GB, wN], f32, tag="mask")
            nc.vector.tensor_scalar(mask_t[:], alphas[p][:], 1.0 / 255.0,
                                    None, Alu.is_ge)
            acs[p] = acpool.tile([GB, wN], f32, tag="ac", name=f"ac{p}")
            prev_dve = nc.vector.tensor_tensor(acs[p][:], amin_t[:],
                                               mask_t[:], Alu.mult)
        for p, pr in enumerate(pairs):
            wN = len(pr) * NPIX
            ls[p] = lpool.tile([GB, wN], f32, tag="l", name=f"l{p}")
            i = nc.scalar.activation(ls[p][:], acs[p][:], Act.Ln, bias=1.0,
                                     scale=-1.0)
            lns.append(i)
            add_dep_helper(i.ins, expA[-1].ins, sync=False,
                           reason="group ACT tables: Ln after phase-A Exps")
        # phase C: cumsum matmul + exp per pair
        for p, pr in enumerate(pairs):
            wN = len(pr) * NPIX
            cum_p = psum.tile([GB, 512], f32, tag="cum", name=f"cum{p}")
            nc.tensor.matmul(cum_p[:, 0:wN], U_t[:], ls[p][:],
                             start=True, stop=True)
            es[p] = epool.tile([GB, wN], f32, tag="e", name=f"e{p}")
            i = nc.scalar.activation(es[p][:], cum_p[:, 0:wN], Act.Exp)
            expC.append(i)
            add_dep_helper(i.ins, lns[-1].ins, sync=False,
                           reason="group ACT tables: Exp after all Ln")
        # phase D: per-job output matmul, copy into staging, DMA per pair
        od = out_d[:].rearrange("p j n -> p (j n)")
        for p, pr in enumerate(pairs):
            for o, j in enumerate(pr):
                out_p = psum.tile([MOUT, NPIX], f32, tag="out")
                nc.tensor.matmul(out_p[:],
                                 wout_t[:, j * MOUT:(j + 1) * MOUT],
                                 es[p][:, o * NPIX:(o + 1) * NPIX],
                                 start=True, stop=True)
                nc.vector.tensor_copy(oall_t[:, j * NPIX:(j + 1) * NPIX],
                                      out_p[:])
            j0, j1 = pr[0], pr[-1] + 1
            nc.sync.dma_start(od[:, j0 * NPIX:j1 * NPIX],
                              oall_t[:, j0 * NPIX:j1 * NPIX])

    nc.compile()
    _NC_CACHE[J] = nc
    return nc


def kernel(means3D, opacities, scales, rotations, sh, bg,
           viewmatrices, projmatrices, camposes, tanfovxs, tanfovys,
           _run_opts=None):
    from concourse.bass_utils import run_bass_kernel_spmd

    means3D = np.asarray(means3D, np.float32)
    opacities = np.asarray(opacities, np.float32)
    scales = np.asarray(scales, np.float32)
    rotations = np.asarray(rotations, np.float32)
    sh = np.asarray(sh, np.float32)
    bg = np.asarray(bg, np.float32)
    viewmatrices = np.asarray(viewmatrices, np.float32)
    projmatrices = np.asarray(projmatrices, np.float32)
    camposes = np.asarray(camposes, np.float32)
    tanfovxs = np.asarray(tanfovxs, np.float32)
    tanfovys = np.asarray(tanfovys, np.float32)

    pps = [_preprocess_one(viewmatrices[b], projmatrices[b], camposes[b],
                           float(tanfovxs[b]), float(tanfovys[b]),
                           means3D, opacities, scales, rotations, sh)
           for b in range(B)]
    groups = _build_groups(pps)

    njob = (len(groups) + NGRP - 1) // NGRP
    J = (njob + NCORES - 1) // NCORES
    # pad with dummy groups (alpha=0 everywhere, no output consumer)
    dummy = dict(key=None, coef=None, wd=None, rfirst=None)
    while len(groups) < J * NCORES * NGRP:
        groups.append(dummy)

    F = _features()
    U8 = np.triu(np.ones((GRAN, GRAN), np.float32))
    U = np.zeros((GB, GB), np.float32)
    for g in range(NGRP):
        U[g * GRAN:(g + 1) * GRAN, g * GRAN:(g + 1) * GRAN] = U8

    # pack per (core, pair): coef [12,GB] (rows 0:6 job a, 6:12 job b)
    NPAIR = (J + 1) // 2
    coef_all = np.zeros((NCORES, NPAIR, 12, GB), np.float32)
    coef_all[:, :, 5, :] = NEG_BIG
    coef_all[:, :, 11, :] = NEG_BIG
    wout_all = np.zeros((NCORES, J, GB, MOUT), np.float32)
    gmeta = {}  # (core, j, g) -> (key, rfirst)
    for i, gr in enumerate(groups):
        cj, g = divmod(i, NGRP)
        c, j = divmod(cj, J)
        if gr['key'] is None:
            continue
        p, half = divmod(j, 2)
        coef_all[c, p, half * 6:(half + 1) * 6,
                 g * GRAN:(g + 1) * GRAN] = gr['coef']
        wout_all[c, j, g * GRAN:(g + 1) * GRAN, g * 5:(g + 1) * 5] = gr['wd']
        gmeta[(c, j, g)] = (gr['key'], gr['rfirst'])

    nc = _build_nc(J)
    in_maps = [{'coef': coef_all[c], 'wout': wout_all[c],
                'feat': F, 'triu': U} for c in range(NCORES)]
    run_opts = dict(_run_opts or {})
    result_sink = run_opts.pop('result_sink', None)
    res = run_bass_kernel_spmd(nc, in_maps, list(range(NCORES)), **run_opts)
    if result_sink is not None:
        result_sink['res'] = res
    outs = [res.results[c]['out'] for c in range(NCORES)]  # [MOUT,J,NPIX]

    # host combine: per (cam, tile) chain chunks in depth order
    nty, ntx = H // TS_Y, W // TS_X
    acc = {}
    for (c, j, g), (key, rfirst) in gmeta.items():
        b, ty, tx, chunk = key
        o = outs[c][g * 5:(g + 1) * 5, j]          # [5, NPIX]
        acc.setdefault((b, ty, tx), []).append((chunk, o, rfirst))
    colors = np.zeros((B, 3, H, W), np.float32)
    invd = np.zeros((B, 1, H, W), np.float32)
    for b in range(B):
        colors[b] = bg[:, None, None]
    for (b, ty, tx), chunks in acc.items():
        chunks.sort(key=lambda t: t[0])
        Csum = np.zeros((4, NPIX), np.float32)
        Trun = np.ones((NPIX,), np.float32)
        for _, o, rfirst in chunks:
            Csum = Csum + Trun[None, :] * (o[0:4] + rfirst[:, None])
            Trun = Trun * o[4]
        ybase, xbase = ty * TS_Y, tx * TS_X
        tilec = (Csum[0:3] + Trun[None, :] * bg[:, None]).reshape(
            3, TS_Y, TS_X)
        colors[b, :, ybase:ybase + TS_Y, xbase:xbase + TS_X] = tilec
        invd[b, 0, ybase:ybase + TS_Y, xbase:xbase + TS_X] = \
            Csum[3].reshape(TS_Y, TS_X)

    radii = np.stack([pp['radii_unsorted'] for pp in pps])
    return colors, invd, radii
